# revision 14
# baseline (speedup 1.0000x reference)
"""AnchorStripeAttention Trainium2 kernel (8 NeuronCores, data-parallel over windows).

Host: window-partition + per-head l2norm + logit-scale fold + CPB bias -> exp(bias)
(multiplicative softmax bias), packed per-window into one 128-partition bf16 blob.
Device (per window): 18 QK matmuls (3-way row-group concurrency, PSUM bank per
row-group), one batched exp, bias multiply (gpsimd bank0 / DVE banks 1-2),
AV1 matmuls (K=128, serial, shared ones-column -> denominators on all parts),
one batched reciprocal, x1 normalize into block-diag x1a, AV2 with x1a as the
STATIONARY operand (LDW 66 cols instead of 128: scores stream as rhs), raw
unnormalized output + stage-2 denominators DMA'd out; final softmax divide on host.

PSUM safety rule: matmuls in different PE row-groups run concurrently, so all
writers of one PSUM bank share a row-group; K=128 matmuls serialize globally.
"""

import math
import sys

import numpy as np

if "/opt/trn_rl_repo" not in sys.path:
    sys.path.insert(0, "/opt/trn_rl_repo")

import concourse.bass as bass  # noqa: E402
import concourse.bacc as bacc  # noqa: E402
import concourse.tile as tile  # noqa: E402
from concourse import mybir  # noqa: E402
from concourse.bass_utils import run_bass_kernel_spmd  # noqa: E402

import ml_dtypes  # noqa: E402

BF16 = np.dtype(ml_dtypes.bfloat16)

NUM_HEADS = 6
DIM = 192
HD = 32
STRIPE = 16
ANCH = 8
B = 2
HS = 256
N1 = STRIPE * STRIPE  # 256 window tokens
N2 = ANCH * ANCH      # 64 anchor tokens
NWIN = 512
NCORES = 8
WPC = NWIN // NCORES  # 64 windows per core
CH = 2                # windows per DMA chunk
LOGIT_MAX = math.log(1.0 / 0.01)

# blob column layout (per window, 128 partitions, bf16); head h = 3f + j
KT_OFF = 0        # rows 32j: col 256f + tok
QT_OFF = 512
ANC_OFF = 1024    # rows 32j: col 64f + anc
VO_OFF = 1152     # col 130j + 65t + c: c=0 ones, c-1=v[h=j], c-33=v[h=j+3]
BLOB_W = VO_OFF + 6 * 65  # 1542

# score layout: bank j (cols 512j..512j+512)
#   S1 (h,t): [all 128 rows,  512j + 128t + 64f : +64]   (tok part, anc col)
#   S2 (h):   [64f:64f+64,    512j + 256 : 512j + 512]   (anc part, tok col)
S_W = 1536

_CACHED = {}


def _build_nc():
    BF = mybir.dt.bfloat16
    F32 = mybir.dt.float32
    EXP = mybir.ActivationFunctionType.Exp

    nc = bacc.Bacc(None)
    blob_d = nc.dram_tensor("blob", [WPC // CH, 128, CH, BLOB_W], BF, kind="ExternalInput")
    expb_d = nc.dram_tensor("expb", [128, S_W], BF, kind="ExternalInput")
    out_d = nc.dram_tensor("out", [WPC // CH, 128, CH, 396], BF, kind="ExternalOutput")

    with tile.TileContext(nc) as tc:
        with (
            tc.tile_pool(name="const", bufs=1) as constp,
            tc.tile_pool(name="inb", bufs=3) as inp,
            tc.tile_pool(name="esp", bufs=3) as esp,
            tc.tile_pool(name="sbp", bufs=3) as sbp,
            tc.tile_pool(name="smallp", bufs=3) as smallp,
            tc.tile_pool(name="outp", bufs=3) as outp,
            tc.tile_pool(name="ps_s", bufs=2, space="PSUM") as ps_s,

        ):
            eb = constp.tile([128, S_W], BF)
            nc.sync.dma_start(eb[:], expb_d[:])
            # persistent block-diagonal AV2 stationary operands (128, 3, 66):
            # rows 0-63 -> cols 0-31 (head j) + ones col 32; rows 64-127 ->
            # cols 33-64 (head j+3) + ones col 65; zeros elsewhere.
            # two sets (window parity) so window w+1 never waits on w's LDW.
            x1as = []
            for s in range(2):
                x1a = constp.tile([128, 3, 66], BF, tag=f"x1a{s}")
                nc.vector.memset(x1a[:], 0.0)
                nc.vector.memset(x1a[0:64, :, 32:33], 1.0)
                nc.vector.memset(x1a[64:128, :, 65:66], 1.0)
                x1as.append(x1a)

            for c in range(WPC // CH):
                bl2 = inp.tile([128, CH, BLOB_W], BF)
                nc.sync.dma_start(bl2[:], blob_d[c])
                of2 = outp.tile([128, CH, 396], BF)
                for s in range(CH):
                    w = c * CH + s
                    bl = bl2[:, s]
                    S = ps_s.tile([128, S_W], F32)
                    es = esp.tile([128, S_W], BF)
                    sb = sbp.tile([128, S_W], BF)
                    # stage-1 QK: a1T half (tok, anc); row-group j -> bank j
                    for t in (0, 1):
                        for h in range(6):
                            j = h % 3
                            f = h // 3
                            nc.tensor.matmul(
                                S[:, 512 * j + 128 * t + 64 * f:512 * j + 128 * t + 64 * f + 64],
                                bl[32 * j:32 * j + 32, KT_OFF + 256 * f + 128 * t:KT_OFF + 256 * f + 128 * t + 128],
                                bl[32 * j:32 * j + 32, ANC_OFF + 64 * f:ANC_OFF + 64 * f + 64],
                                start=True, stop=True,
                                tile_position=(32 * j, 0),
                            )
                    # stage-2 QK: a2T (anc, tok); pair (h, h+3) stacks in bank j
                    for h in range(6):
                        j = h % 3
                        f = h // 3
                        nc.tensor.matmul(
                            S[64 * f:64 * f + 64, 512 * j + 256:512 * j + 512],
                            bl[32 * j:32 * j + 32, ANC_OFF + 64 * f:ANC_OFF + 64 * f + 64],
                            bl[32 * j:32 * j + 32, QT_OFF + 256 * f:QT_OFF + 256 * f + 256],
                            start=True, stop=True,
                            tile_position=(32 * j, 64 * f),
                        )
                    # two-call exp (banks 1-2 first, then bank 0) + multiplicative
                    # bias: DVE multiplies banks 1-2, gpsimd bank 0. AV order
                    # (1, 2, 0) starts on the DVE-produced banks.
                    nc.scalar.activation(es[:, 512:1536], S[:, 512:1536], EXP)
                    nc.vector.tensor_tensor(
                        sb[:, 512:1536], es[:, 512:1536], eb[:, 512:1536], mybir.AluOpType.mult)
                    nc.scalar.activation(es[:, 0:512], S[:, 0:512], EXP)
                    nc.gpsimd.tensor_tensor(
                        sb[:, 0:512], es[:, 0:512], eb[:, 0:512], mybir.AluOpType.mult)
                    # stage-1 AV merged pairs (K=128, serial): one MM per (j, t)
                    # computes both heads 3f+j; col 0 of the VO operand is all-ones
                    # so xo1 col 65j holds the softmax denominator on ALL partitions.
                    # PSUM reuse: the scores are dead in PSUM once exp has read
                    # them, so xo1 (cols 512j..+65) and xo2 (cols 512j+256+66t)
                    # live inside the S tile -- 6 banks total, clean 2-parity.
                    Sv = S[:].rearrange("p (j b) -> p j b", j=3)
                    for j in (1, 2, 0):
                        for t in (0, 1):
                            nc.tensor.matmul(
                                S[:, 512 * j:512 * j + 65],
                                sb[:, 512 * j + 128 * t:512 * j + 128 * t + 128],
                                bl[:, VO_OFF + 130 * j + 65 * t:VO_OFF + 130 * j + 65 * t + 65],
                                start=(t == 0), stop=(t == 1),
                            )
                    rd1 = smallp.tile([128, 3, 1], mybir.dt.float32)
                    nc.vector.reciprocal(rd1[:], Sv[:, :, 0:1])
                    # normalize x1u into the block-diag x1a (ones cols stay 1.0
                    # for the stage-2 denominator)
                    x1a = x1as[w % 2]
                    nc.vector.tensor_tensor(
                        x1a[0:64, :, 0:32], Sv[0:64, :, 1:33],
                        rd1[0:64, :, 0:1].to_broadcast((64, 3, 32)), mybir.AluOpType.mult)
                    nc.vector.tensor_tensor(
                        x1a[64:128, :, 33:65], Sv[64:128, :, 33:65],
                        rd1[64:128, :, 0:1].to_broadcast((64, 3, 32)), mybir.AluOpType.mult)
                    # stage-2 AV merged pairs (K=128 over stacked anchors,
                    # block-diag rhs); output overwrites the dead S2 score region
                    for j in (1, 2, 0):
                        for t in (0, 1):
                            nc.tensor.matmul(
                                S[:, 512 * j + 256 + 66 * t:512 * j + 256 + 66 * t + 66],
                                sb[:, 512 * j + 256 + 128 * t:512 * j + 256 + 128 * t + 128],
                                x1a[:, j, :],
                                start=True, stop=True,
                            )
                    nc.vector.tensor_copy(
                        of2[:, s].rearrange("p (j c) -> p j c", j=3),
                        Sv[:, :, 256:388])
                nc.sync.dma_start(out_d[c], of2[:])
    return nc


def _get_nc():
    if "nc" not in _CACHED:
        nc = _build_nc()
        nc.compile()
        _CACHED["nc"] = nc
    return _CACHED["nc"]


def _l2n(x):
    n = np.sqrt((x * x).sum(-1, keepdims=True))
    return x / np.maximum(n, 1e-12)


def _prepare(qkv, anchor, table, logit_scale1, cpb1_w1, cpb1_b1, cpb1_w2,
             logit_scale2, cpb2_w1, cpb2_b1, cpb2_w2, index_a2w, index_w2a):
    f32 = np.float32
    t2 = np.asarray(table, f32).reshape(-1, 2)
    bt1 = np.maximum(t2 @ np.asarray(cpb1_w1, f32) + np.asarray(cpb1_b1, f32), 0.0) @ np.asarray(cpb1_w2, f32)
    bt2 = np.maximum(t2 @ np.asarray(cpb2_w1, f32) + np.asarray(cpb2_b1, f32), 0.0) @ np.asarray(cpb2_w2, f32)
    ia = np.asarray(index_a2w).astype(np.int64).reshape(-1)
    iw = np.asarray(index_w2a).astype(np.int64).reshape(-1)
    b1 = 16.0 / (1.0 + np.exp(-bt1[ia]))
    b1 = b1.reshape(N2, N1, NUM_HEADS).transpose(2, 0, 1)  # (6, anc, tok)
    b2 = 16.0 / (1.0 + np.exp(-bt2[iw]))
    b2 = b2.reshape(N1, N2, NUM_HEADS).transpose(2, 0, 1)  # (6, tok, anc)

    expb = np.zeros((128, S_W), f32)
    for h in range(6):
        j = h % 3
        e = h // 3
        for t in (0, 1):
            expb[:, 512 * j + 128 * t + 64 * e:512 * j + 128 * t + 64 * e + 64] = \
                np.exp(b1[h, :, 128 * t:128 * (t + 1)]).T
        expb[64 * e:64 * e + 64, 512 * j + 256:512 * j + 512] = np.exp(b2[h]).T

    s1 = np.exp(np.minimum(np.asarray(logit_scale1, f32).reshape(NUM_HEADS), LOGIT_MAX))
    s2 = np.exp(np.minimum(np.asarray(logit_scale2, f32).reshape(NUM_HEADS), LOGIT_MAX))

    qkv4 = np.ascontiguousarray(np.asarray(qkv, f32).reshape(B, 16, STRIPE, 16, STRIPE, 3 * DIM)
                                .transpose(0, 1, 3, 2, 4, 5)).reshape(NWIN, N1, 3 * DIM)
    q = qkv4[:, :, :DIM].reshape(NWIN, N1, NUM_HEADS, HD)
    k = qkv4[:, :, DIM:2 * DIM].reshape(NWIN, N1, NUM_HEADS, HD)
    v = qkv4[:, :, 2 * DIM:].reshape(NWIN, N1, NUM_HEADS, HD)
    anc4 = np.ascontiguousarray(np.asarray(anchor, f32).reshape(B, 16, ANCH, 16, ANCH, DIM)
                                .transpose(0, 1, 3, 2, 4, 5)).reshape(NWIN, N2, NUM_HEADS, HD)

    kn = _l2n(k) * s1[None, None, :, None]
    qn = _l2n(q) * s2[None, None, :, None]
    an = _l2n(anc4)

    blob = np.zeros((NWIN, 128, BLOB_W), BF16)
    for h in range(6):
        r = 32 * (h % 3)
        cb = h // 3
        blob[:, r:r + 32, KT_OFF + 256 * cb:KT_OFF + 256 * cb + 256] = kn[:, :, h, :].transpose(0, 2, 1)
        blob[:, r:r + 32, QT_OFF + 256 * cb:QT_OFF + 256 * cb + 256] = qn[:, :, h, :].transpose(0, 2, 1)
        blob[:, r:r + 32, ANC_OFF + 64 * cb:ANC_OFF + 64 * cb + 64] = an[:, :, h, :].transpose(0, 2, 1)
    for j in range(3):
        for t in (0, 1):
            c0 = VO_OFF + 130 * j + 65 * t
            blob[:, :, c0] = 1.0
            blob[:, :, c0 + 1:c0 + 33] = v[:, 128 * t:128 * (t + 1), j, :]
            blob[:, :, c0 + 33:c0 + 65] = v[:, 128 * t:128 * (t + 1), j + 3, :]

    in_maps = []
    expb_bf = expb.astype(BF16)
    blob = blob.reshape(NWIN // CH, CH, 128, BLOB_W).transpose(0, 2, 1, 3)
    blob = np.ascontiguousarray(blob)  # (NWIN/CH, 128, CH, BLOB_W)
    cpc = WPC // CH
    for c in range(NCORES):
        in_maps.append({
            "blob": blob[c * cpc:(c + 1) * cpc],
            "expb": expb_bf,
        })
    return in_maps


def _assemble(results):
    # out: (WPC/CH, 128, CH, 396) per core; col = 132j + 66t + c
    outw = np.concatenate(
        [np.asarray(r["out"], np.float32).transpose(0, 2, 1, 3)
         .reshape(WPC, 128, 3, 2, 66) for r in results], axis=0)
    x = np.empty((NWIN, 2, 128, NUM_HEADS, HD), np.float32)  # (w, t, part, h, d)
    for j in range(3):
        x[:, :, :, j, :] = (outw[:, :, j, :, 0:32] /
                            outw[:, :, j, :, 32:33]).transpose(0, 2, 1, 3)
        x[:, :, :, j + 3, :] = (outw[:, :, j, :, 33:65] /
                                outw[:, :, j, :, 65:66]).transpose(0, 2, 1, 3)
    full = x.reshape(NWIN, N1, DIM)
    img = full.reshape(B, 16, 16, STRIPE, STRIPE, DIM).transpose(0, 1, 3, 2, 4, 5)
    return np.ascontiguousarray(img).reshape(B, HS * HS, DIM)


def _run(inputs, trace=False, trace_kwargs=None):
    in_maps = _prepare(
        inputs["qkv"], inputs["anchor"], inputs["table"],
        inputs["logit_scale1"], inputs["cpb1_w1"], inputs["cpb1_b1"], inputs["cpb1_w2"],
        inputs["logit_scale2"], inputs["cpb2_w1"], inputs["cpb2_b1"], inputs["cpb2_w2"],
        inputs["index_a2w"], inputs["index_w2a"],
    )
    nc = _get_nc()
    res = run_bass_kernel_spmd(
        nc, in_maps, core_ids=list(range(NCORES)),
        trace=trace, **(trace_kwargs or {}),
    )
    out = _assemble(res.results)
    return out, res


def kernel(**inputs):
    out, _ = _run(inputs, trace=False)
    return out


# revision 16
# speedup vs baseline: 1.0922x; 1.0922x over previous
"""AnchorStripeAttention Trainium2 kernel (8 NeuronCores, data-parallel over windows).

Host: window-partition + per-head l2norm + logit-scale fold + CPB bias -> exp(bias)
(multiplicative softmax bias), packed per-window into one 128-partition bf16 blob.
Device (per window): 18 QK matmuls (3-way row-group concurrency, PSUM bank per
row-group), one batched exp, bias multiply (gpsimd bank0 / DVE banks 1-2),
AV1 matmuls (K=128, serial, shared ones-column -> denominators on all parts),
one batched reciprocal, x1 normalize into block-diag x1a, AV2 with x1a as the
STATIONARY operand (LDW 66 cols instead of 128: scores stream as rhs), raw
unnormalized output + stage-2 denominators DMA'd out; final softmax divide on host.

PSUM safety rule: matmuls in different PE row-groups run concurrently, so all
writers of one PSUM bank share a row-group; K=128 matmuls serialize globally.
"""

import math
import sys

import numpy as np

if "/opt/trn_rl_repo" not in sys.path:
    sys.path.insert(0, "/opt/trn_rl_repo")

import concourse.bass as bass  # noqa: E402
import concourse.bacc as bacc  # noqa: E402
import concourse.tile as tile  # noqa: E402
from concourse import mybir  # noqa: E402
from concourse.bass_utils import run_bass_kernel_spmd  # noqa: E402

import ml_dtypes  # noqa: E402

BF16 = np.dtype(ml_dtypes.bfloat16)

NUM_HEADS = 6
DIM = 192
HD = 32
STRIPE = 16
ANCH = 8
B = 2
HS = 256
N1 = STRIPE * STRIPE  # 256 window tokens
N2 = ANCH * ANCH      # 64 anchor tokens
NWIN = 512
NCORES = 8
WPC = NWIN // NCORES  # 64 windows per core
CH = 2                # windows per DMA chunk
LOGIT_MAX = math.log(1.0 / 0.01)

# blob column layout (per window, 128 partitions, bf16); head h = 3f + j
KT_OFF = 0        # rows 32j: col 256f + tok
QT_OFF = 512
ANC_OFF = 1024    # rows 32j: col 64f + anc
VO_OFF = 1152     # col 130j + 65t + c: c=0 ones, c-1=v[h=j], c-33=v[h=j+3]
BLOB_W = VO_OFF + 6 * 65  # 1542

# score layout: bank j (cols 512j..512j+512)
#   S1 (h,t): [all 128 rows,  512j + 128t + 64f : +64]   (tok part, anc col)
#   S2 (h):   [64f:64f+64,    512j + 256 : 512j + 512]   (anc part, tok col)
S_W = 1536

_CACHED = {}


def _build_nc():
    BF = mybir.dt.bfloat16
    F32 = mybir.dt.float32
    EXP = mybir.ActivationFunctionType.Exp

    nc = bacc.Bacc(None)
    blob_d = nc.dram_tensor("blob", [WPC // CH, 128, CH, BLOB_W], BF, kind="ExternalInput")
    expb_d = nc.dram_tensor("expb", [128, S_W], BF, kind="ExternalInput")
    out_d = nc.dram_tensor("out", [WPC // CH, 128, CH, 396], BF, kind="ExternalOutput")

    with tile.TileContext(nc) as tc:
        with (
            tc.tile_pool(name="const", bufs=1) as constp,
            tc.tile_pool(name="inb", bufs=3) as inp,
            tc.tile_pool(name="esp", bufs=3) as esp,
            tc.tile_pool(name="sbp", bufs=3) as sbp,
            tc.tile_pool(name="smallp", bufs=3) as smallp,
            tc.tile_pool(name="outp", bufs=3) as outp,
            tc.tile_pool(name="ps_s", bufs=2, space="PSUM") as ps_s,
            tc.tile_pool(name="ps_x2", bufs=2, space="PSUM") as ps_x2,

        ):
            eb = constp.tile([128, S_W], BF)
            nc.sync.dma_start(eb[:], expb_d[:])
            # persistent block-diagonal AV2 stationary operands (128, 3, 66):
            # rows 0-63 -> cols 0-31 (head j) + ones col 32; rows 64-127 ->
            # cols 33-64 (head j+3) + ones col 65; zeros elsewhere.
            # two sets (window parity) so window w+1 never waits on w's LDW.
            x1as = []
            for s in range(2):
                x1a = constp.tile([128, 3, 66], BF, tag=f"x1a{s}")
                nc.vector.memset(x1a[:], 0.0)
                nc.vector.memset(x1a[0:64, :, 32:33], 1.0)
                nc.vector.memset(x1a[64:128, :, 65:66], 1.0)
                x1as.append(x1a)

            for c in range(WPC // CH):
                bl2 = inp.tile([128, CH, BLOB_W], BF)
                nc.sync.dma_start(bl2[:], blob_d[c])
                of2 = outp.tile([128, CH, 396], BF)
                for s in range(CH):
                    w = c * CH + s
                    bl = bl2[:, s]
                    S = ps_s.tile([128, S_W], F32)
                    es = esp.tile([128, S_W], BF)
                    sb = sbp.tile([128, S_W], BF)
                    # stage-1 QK: a1T half (tok, anc); row-group j -> bank j
                    for t in (0, 1):
                        for h in range(6):
                            j = h % 3
                            f = h // 3
                            nc.tensor.matmul(
                                S[:, 512 * j + 128 * t + 64 * f:512 * j + 128 * t + 64 * f + 64],
                                bl[32 * j:32 * j + 32, KT_OFF + 256 * f + 128 * t:KT_OFF + 256 * f + 128 * t + 128],
                                bl[32 * j:32 * j + 32, ANC_OFF + 64 * f:ANC_OFF + 64 * f + 64],
                                start=True, stop=True,
                                tile_position=(32 * j, 0),
                            )
                    # stage-2 QK: a2T (anc, tok); pair (h, h+3) stacks in bank j
                    for h in range(6):
                        j = h % 3
                        f = h // 3
                        nc.tensor.matmul(
                            S[64 * f:64 * f + 64, 512 * j + 256:512 * j + 512],
                            bl[32 * j:32 * j + 32, ANC_OFF + 64 * f:ANC_OFF + 64 * f + 64],
                            bl[32 * j:32 * j + 32, QT_OFF + 256 * f:QT_OFF + 256 * f + 256],
                            start=True, stop=True,
                            tile_position=(32 * j, 64 * f),
                        )
                    # two-call exp (banks 1-2 first, then bank 0) + multiplicative
                    # bias: DVE multiplies banks 1-2, gpsimd bank 0. AV order
                    # (1, 2, 0) starts on the DVE-produced banks.
                    nc.scalar.activation(es[:, 512:1536], S[:, 512:1536], EXP)
                    nc.vector.tensor_tensor(
                        sb[:, 512:1536], es[:, 512:1536], eb[:, 512:1536], mybir.AluOpType.mult)
                    nc.scalar.activation(es[:, 0:512], S[:, 0:512], EXP)
                    nc.gpsimd.tensor_tensor(
                        sb[:, 0:512], es[:, 0:512], eb[:, 0:512], mybir.AluOpType.mult)
                    # stage-1 AV merged pairs (K=128, serial): one MM per (j, t)
                    # computes both heads 3f+j; col 0 of the VO operand is all-ones
                    # so xo1 col 65j holds the softmax denominator on ALL partitions.
                    # PSUM reuse: the scores are dead in PSUM once exp has read
                    # them, so xo1 (cols 512j..+65) and xo2 (cols 512j+256+66t)
                    # live inside the S tile -- 6 banks total, clean 2-parity.
                    Sv = S[:].rearrange("p (j b) -> p j b", j=3)
                    for j in (1, 2, 0):
                        for t in (0, 1):
                            nc.tensor.matmul(
                                S[:, 512 * j:512 * j + 65],
                                sb[:, 512 * j + 128 * t:512 * j + 128 * t + 128],
                                bl[:, VO_OFF + 130 * j + 65 * t:VO_OFF + 130 * j + 65 * t + 65],
                                start=(t == 0), stop=(t == 1),
                            )
                    rd1 = smallp.tile([128, 3, 1], mybir.dt.float32)
                    nc.vector.reciprocal(rd1[:], Sv[:, :, 0:1])
                    # normalize x1u into the block-diag x1a (ones cols stay 1.0
                    # for the stage-2 denominator)
                    x1a = x1as[w % 2]
                    nc.vector.tensor_tensor(
                        x1a[0:64, :, 0:32], Sv[0:64, :, 1:33],
                        rd1[0:64, :, 0:1].to_broadcast((64, 3, 32)), mybir.AluOpType.mult)
                    nc.vector.tensor_tensor(
                        x1a[64:128, :, 33:65], Sv[64:128, :, 33:65],
                        rd1[64:128, :, 0:1].to_broadcast((64, 3, 32)), mybir.AluOpType.mult)
                    # stage-2 AV merged pairs (K=128 over stacked anchors,
                    # block-diag rhs); own double-buffered bank so the S tile
                    # frees at x1norm and window w+2's QK can start early
                    xo2 = ps_x2.tile([128, 3, 2, 66], F32)
                    for j in (1, 2, 0):
                        for t in (0, 1):
                            nc.tensor.matmul(
                                xo2[:, j, t, :],
                                sb[:, 512 * j + 256 + 128 * t:512 * j + 256 + 128 * t + 128],
                                x1a[:, j, :],
                                start=True, stop=True,
                            )
                    nc.vector.tensor_copy(
                        of2[:, s].rearrange("p (j t c) -> p j t c", j=3, t=2), xo2[:])
                nc.sync.dma_start(out_d[c], of2[:])
    return nc


def _get_nc():
    if "nc" not in _CACHED:
        nc = _build_nc()
        nc.compile()
        _CACHED["nc"] = nc
    return _CACHED["nc"]


def _l2n(x):
    n = np.sqrt((x * x).sum(-1, keepdims=True))
    return x / np.maximum(n, 1e-12)


def _prepare(qkv, anchor, table, logit_scale1, cpb1_w1, cpb1_b1, cpb1_w2,
             logit_scale2, cpb2_w1, cpb2_b1, cpb2_w2, index_a2w, index_w2a):
    f32 = np.float32
    t2 = np.asarray(table, f32).reshape(-1, 2)
    bt1 = np.maximum(t2 @ np.asarray(cpb1_w1, f32) + np.asarray(cpb1_b1, f32), 0.0) @ np.asarray(cpb1_w2, f32)
    bt2 = np.maximum(t2 @ np.asarray(cpb2_w1, f32) + np.asarray(cpb2_b1, f32), 0.0) @ np.asarray(cpb2_w2, f32)
    ia = np.asarray(index_a2w).astype(np.int64).reshape(-1)
    iw = np.asarray(index_w2a).astype(np.int64).reshape(-1)
    b1 = 16.0 / (1.0 + np.exp(-bt1[ia]))
    b1 = b1.reshape(N2, N1, NUM_HEADS).transpose(2, 0, 1)  # (6, anc, tok)
    b2 = 16.0 / (1.0 + np.exp(-bt2[iw]))
    b2 = b2.reshape(N1, N2, NUM_HEADS).transpose(2, 0, 1)  # (6, tok, anc)

    expb = np.zeros((128, S_W), f32)
    for h in range(6):
        j = h % 3
        e = h // 3
        for t in (0, 1):
            expb[:, 512 * j + 128 * t + 64 * e:512 * j + 128 * t + 64 * e + 64] = \
                np.exp(b1[h, :, 128 * t:128 * (t + 1)]).T
        expb[64 * e:64 * e + 64, 512 * j + 256:512 * j + 512] = np.exp(b2[h]).T

    s1 = np.exp(np.minimum(np.asarray(logit_scale1, f32).reshape(NUM_HEADS), LOGIT_MAX))
    s2 = np.exp(np.minimum(np.asarray(logit_scale2, f32).reshape(NUM_HEADS), LOGIT_MAX))

    qkv4 = np.ascontiguousarray(np.asarray(qkv, f32).reshape(B, 16, STRIPE, 16, STRIPE, 3 * DIM)
                                .transpose(0, 1, 3, 2, 4, 5)).reshape(NWIN, N1, 3 * DIM)
    q = qkv4[:, :, :DIM].reshape(NWIN, N1, NUM_HEADS, HD)
    k = qkv4[:, :, DIM:2 * DIM].reshape(NWIN, N1, NUM_HEADS, HD)
    v = qkv4[:, :, 2 * DIM:].reshape(NWIN, N1, NUM_HEADS, HD)
    anc4 = np.ascontiguousarray(np.asarray(anchor, f32).reshape(B, 16, ANCH, 16, ANCH, DIM)
                                .transpose(0, 1, 3, 2, 4, 5)).reshape(NWIN, N2, NUM_HEADS, HD)

    kn = _l2n(k) * s1[None, None, :, None]
    qn = _l2n(q) * s2[None, None, :, None]
    an = _l2n(anc4)

    blob = np.zeros((NWIN, 128, BLOB_W), BF16)
    for h in range(6):
        r = 32 * (h % 3)
        cb = h // 3
        blob[:, r:r + 32, KT_OFF + 256 * cb:KT_OFF + 256 * cb + 256] = kn[:, :, h, :].transpose(0, 2, 1)
        blob[:, r:r + 32, QT_OFF + 256 * cb:QT_OFF + 256 * cb + 256] = qn[:, :, h, :].transpose(0, 2, 1)
        blob[:, r:r + 32, ANC_OFF + 64 * cb:ANC_OFF + 64 * cb + 64] = an[:, :, h, :].transpose(0, 2, 1)
    for j in range(3):
        for t in (0, 1):
            c0 = VO_OFF + 130 * j + 65 * t
            blob[:, :, c0] = 1.0
            blob[:, :, c0 + 1:c0 + 33] = v[:, 128 * t:128 * (t + 1), j, :]
            blob[:, :, c0 + 33:c0 + 65] = v[:, 128 * t:128 * (t + 1), j + 3, :]

    in_maps = []
    expb_bf = expb.astype(BF16)
    blob = blob.reshape(NWIN // CH, CH, 128, BLOB_W).transpose(0, 2, 1, 3)
    blob = np.ascontiguousarray(blob)  # (NWIN/CH, 128, CH, BLOB_W)
    cpc = WPC // CH
    for c in range(NCORES):
        in_maps.append({
            "blob": blob[c * cpc:(c + 1) * cpc],
            "expb": expb_bf,
        })
    return in_maps


def _assemble(results):
    # out: (WPC/CH, 128, CH, 396) per core; col = 132j + 66t + c
    outw = np.concatenate(
        [np.asarray(r["out"], np.float32).transpose(0, 2, 1, 3)
         .reshape(WPC, 128, 3, 2, 66) for r in results], axis=0)
    x = np.empty((NWIN, 2, 128, NUM_HEADS, HD), np.float32)  # (w, t, part, h, d)
    for j in range(3):
        x[:, :, :, j, :] = (outw[:, :, j, :, 0:32] /
                            outw[:, :, j, :, 32:33]).transpose(0, 2, 1, 3)
        x[:, :, :, j + 3, :] = (outw[:, :, j, :, 33:65] /
                                outw[:, :, j, :, 65:66]).transpose(0, 2, 1, 3)
    full = x.reshape(NWIN, N1, DIM)
    img = full.reshape(B, 16, 16, STRIPE, STRIPE, DIM).transpose(0, 1, 3, 2, 4, 5)
    return np.ascontiguousarray(img).reshape(B, HS * HS, DIM)


def _run(inputs, trace=False, trace_kwargs=None):
    in_maps = _prepare(
        inputs["qkv"], inputs["anchor"], inputs["table"],
        inputs["logit_scale1"], inputs["cpb1_w1"], inputs["cpb1_b1"], inputs["cpb1_w2"],
        inputs["logit_scale2"], inputs["cpb2_w1"], inputs["cpb2_b1"], inputs["cpb2_w2"],
        inputs["index_a2w"], inputs["index_w2a"],
    )
    nc = _get_nc()
    res = run_bass_kernel_spmd(
        nc, in_maps, core_ids=list(range(NCORES)),
        trace=trace, **(trace_kwargs or {}),
    )
    out = _assemble(res.results)
    return out, res


def kernel(**inputs):
    out, _ = _run(inputs, trace=False)
    return out


# revision 17
# speedup vs baseline: 1.3152x; 1.2043x over previous
"""AnchorStripeAttention Trainium2 kernel (8 NeuronCores, data-parallel over windows).

Host: window-partition + per-head l2norm + logit-scale fold + CPB bias -> exp(bias)
(multiplicative softmax bias), packed per-window into one 128-partition bf16 blob.
Device (per window): 18 QK matmuls (3-way row-group concurrency, PSUM bank per
row-group), one batched exp, bias multiply (gpsimd bank0 / DVE banks 1-2),
AV1 matmuls (K=128, serial, shared ones-column -> denominators on all parts),
one batched reciprocal, x1 normalize into block-diag x1a, AV2 with x1a as the
STATIONARY operand (LDW 66 cols instead of 128: scores stream as rhs), raw
unnormalized output + stage-2 denominators DMA'd out; final softmax divide on host.

PSUM safety rule: matmuls in different PE row-groups run concurrently, so all
writers of one PSUM bank share a row-group; K=128 matmuls serialize globally.
"""

import math
import sys

import numpy as np

if "/opt/trn_rl_repo" not in sys.path:
    sys.path.insert(0, "/opt/trn_rl_repo")

import concourse.bass as bass  # noqa: E402
import concourse.bacc as bacc  # noqa: E402
import concourse.tile as tile  # noqa: E402
from concourse import mybir  # noqa: E402
from concourse.bass_utils import run_bass_kernel_spmd  # noqa: E402

import ml_dtypes  # noqa: E402

BF16 = np.dtype(ml_dtypes.bfloat16)

NUM_HEADS = 6
DIM = 192
HD = 32
STRIPE = 16
ANCH = 8
B = 2
HS = 256
N1 = STRIPE * STRIPE  # 256 window tokens
N2 = ANCH * ANCH      # 64 anchor tokens
NWIN = 512
NCORES = 8
WPC = NWIN // NCORES  # 64 windows per core
CH = 2                # windows per DMA chunk
LOGIT_MAX = math.log(1.0 / 0.01)

# blob column layout (per window, 128 partitions, bf16); head h = 3f + j
KT_OFF = 0        # rows 32j: col 256f + tok
QT_OFF = 512
ANC_OFF = 1024    # rows 32j: col 64f + anc
VO_OFF = 1152     # col 130j + 65t + c: c=0 ones, c-1=v[h=j], c-33=v[h=j+3]
BLOB_W = VO_OFF + 6 * 65  # 1542

# score layout: bank j (cols 512j..512j+512)
#   S1 (h,t): [all 128 rows,  512j + 128t + 64f : +64]   (tok part, anc col)
#   S2 (h):   [64f:64f+64,    512j + 256 : 512j + 512]   (anc part, tok col)
S_W = 1536

_CACHED = {}


def _build_nc():
    BF = mybir.dt.bfloat16
    F32 = mybir.dt.float32
    EXP = mybir.ActivationFunctionType.Exp

    nc = bacc.Bacc(None)
    blob_d = nc.dram_tensor("blob", [WPC // CH, 128, CH, BLOB_W], BF, kind="ExternalInput")
    expb_d = nc.dram_tensor("expb", [128, S_W], BF, kind="ExternalInput")
    out_d = nc.dram_tensor("out", [WPC // CH, 128, CH, 396], BF, kind="ExternalOutput")

    with tile.TileContext(nc) as tc:
        with (
            tc.tile_pool(name="const", bufs=1) as constp,
            tc.tile_pool(name="inb", bufs=3) as inp,
            tc.tile_pool(name="esp", bufs=3) as esp,
            tc.tile_pool(name="sbp", bufs=3) as sbp,
            tc.tile_pool(name="smallp", bufs=3) as smallp,
            tc.tile_pool(name="outp", bufs=3) as outp,
            tc.tile_pool(name="ps_s", bufs=2, space="PSUM") as ps_s,
            tc.tile_pool(name="ps_x2", bufs=2, space="PSUM") as ps_x2,

        ):
            eb = constp.tile([128, S_W], BF)
            nc.sync.dma_start(eb[:], expb_d[:])
            # persistent block-diagonal AV2 stationary operands (128, 3, 66):
            # rows 0-63 -> cols 0-31 (head j) + ones col 32; rows 64-127 ->
            # cols 33-64 (head j+3) + ones col 65; zeros elsewhere.
            # two sets (window parity) so window w+1 never waits on w's LDW.
            x1as = []
            for s in range(2):
                x1a = constp.tile([128, 3, 66], BF, tag=f"x1a{s}")
                nc.vector.memset(x1a[:], 0.0)
                nc.vector.memset(x1a[0:64, :, 32:33], 1.0)
                nc.vector.memset(x1a[64:128, :, 65:66], 1.0)
                x1as.append(x1a)

            # software-pipelined emission: per iteration emit QK+exp of window w,
            # then the AV tail of window w-1, then the bias-mults of w -- so the
            # Tensor queue interleaves [QK(w), AV(w-1), QK(w+1), AV(w), ...] and
            # never stalls behind the exp -> mult chain of its own window.
            state = None
            bl2 = of2 = None
            for w in range(WPC + 1):
                if w < WPC:
                    s = w % CH
                    if s == 0:
                        bl2 = inp.tile([128, CH, BLOB_W], BF)
                        nc.sync.dma_start(bl2[:], blob_d[w // CH])
                        of2 = outp.tile([128, CH, 396], BF)
                    bl = bl2[:, s]
                    S = ps_s.tile([128, S_W], F32)
                    es = esp.tile([128, S_W], BF)
                    sb = sbp.tile([128, S_W], BF)
                    # stage-1 QK: a1T half (tok, anc); row-group j -> bank j
                    for t in (0, 1):
                        for h in range(6):
                            j = h % 3
                            f = h // 3
                            nc.tensor.matmul(
                                S[:, 512 * j + 128 * t + 64 * f:512 * j + 128 * t + 64 * f + 64],
                                bl[32 * j:32 * j + 32, KT_OFF + 256 * f + 128 * t:KT_OFF + 256 * f + 128 * t + 128],
                                bl[32 * j:32 * j + 32, ANC_OFF + 64 * f:ANC_OFF + 64 * f + 64],
                                start=True, stop=True,
                                tile_position=(32 * j, 0),
                            )
                    # stage-2 QK: a2T (anc, tok); pair (h, h+3) stacks in bank j
                    for h in range(6):
                        j = h % 3
                        f = h // 3
                        nc.tensor.matmul(
                            S[64 * f:64 * f + 64, 512 * j + 256:512 * j + 512],
                            bl[32 * j:32 * j + 32, ANC_OFF + 64 * f:ANC_OFF + 64 * f + 64],
                            bl[32 * j:32 * j + 32, QT_OFF + 256 * f:QT_OFF + 256 * f + 256],
                            start=True, stop=True,
                            tile_position=(32 * j, 64 * f),
                        )
                    # two-call exp (banks 1-2 feed the DVE mult, bank 0 gpsimd)
                    nc.scalar.activation(es[:, 512:1536], S[:, 512:1536], EXP)
                    nc.scalar.activation(es[:, 0:512], S[:, 0:512], EXP)
                    cur = (S, sb, es, bl, of2, s, w)
                if state is not None:
                    S_, sb_, es_, bl_, of2_, s_, w_ = state
                    Sv_ = S_[:].rearrange("p (j b) -> p j b", j=3)
                    # stage-1 AV merged pairs (K=128, serial): one MM per (j, t)
                    # computes both heads 3f+j; col 0 of the VO operand is ones
                    # so xo1 col 65j holds the denominator on ALL partitions.
                    # xo1 reuses the dead S1 score region (cols 512j..+65).
                    for j in (1, 2, 0):
                        for t in (0, 1):
                            nc.tensor.matmul(
                                S_[:, 512 * j:512 * j + 65],
                                sb_[:, 512 * j + 128 * t:512 * j + 128 * t + 128],
                                bl_[:, VO_OFF + 130 * j + 65 * t:VO_OFF + 130 * j + 65 * t + 65],
                                start=(t == 0), stop=(t == 1),
                            )
                    rd1 = smallp.tile([128, 3, 1], mybir.dt.float32)
                    nc.vector.reciprocal(rd1[:], Sv_[:, :, 0:1])
                    x1a = x1as[w_ % 2]
                    nc.vector.tensor_tensor(
                        x1a[0:64, :, 0:32], Sv_[0:64, :, 1:33],
                        rd1[0:64, :, 0:1].to_broadcast((64, 3, 32)), mybir.AluOpType.mult)
                    nc.vector.tensor_tensor(
                        x1a[64:128, :, 33:65], Sv_[64:128, :, 33:65],
                        rd1[64:128, :, 0:1].to_broadcast((64, 3, 32)), mybir.AluOpType.mult)
                    # stage-2 AV merged pairs; own double-buffered bank
                    xo2 = ps_x2.tile([128, 3, 2, 66], F32)
                    for j in (1, 2, 0):
                        for t in (0, 1):
                            nc.tensor.matmul(
                                xo2[:, j, t, :],
                                sb_[:, 512 * j + 256 + 128 * t:512 * j + 256 + 128 * t + 128],
                                x1a[:, j, :],
                                start=True, stop=True,
                            )
                    nc.vector.tensor_copy(
                        of2_[:, s_].rearrange("p (j t c) -> p j t c", j=3, t=2), xo2[:])
                    if s_ == CH - 1:
                        nc.sync.dma_start(out_d[w_ // CH], of2_[:])
                if w < WPC:
                    nc.vector.tensor_tensor(
                        sb[:, 512:1536], es[:, 512:1536], eb[:, 512:1536], mybir.AluOpType.mult)
                    nc.gpsimd.tensor_tensor(
                        sb[:, 0:512], es[:, 0:512], eb[:, 0:512], mybir.AluOpType.mult)
                    state = cur
    return nc


def _get_nc():
    if "nc" not in _CACHED:
        nc = _build_nc()
        nc.compile()
        _CACHED["nc"] = nc
    return _CACHED["nc"]


def _l2n(x):
    n = np.sqrt((x * x).sum(-1, keepdims=True))
    return x / np.maximum(n, 1e-12)


def _prepare(qkv, anchor, table, logit_scale1, cpb1_w1, cpb1_b1, cpb1_w2,
             logit_scale2, cpb2_w1, cpb2_b1, cpb2_w2, index_a2w, index_w2a):
    f32 = np.float32
    t2 = np.asarray(table, f32).reshape(-1, 2)
    bt1 = np.maximum(t2 @ np.asarray(cpb1_w1, f32) + np.asarray(cpb1_b1, f32), 0.0) @ np.asarray(cpb1_w2, f32)
    bt2 = np.maximum(t2 @ np.asarray(cpb2_w1, f32) + np.asarray(cpb2_b1, f32), 0.0) @ np.asarray(cpb2_w2, f32)
    ia = np.asarray(index_a2w).astype(np.int64).reshape(-1)
    iw = np.asarray(index_w2a).astype(np.int64).reshape(-1)
    b1 = 16.0 / (1.0 + np.exp(-bt1[ia]))
    b1 = b1.reshape(N2, N1, NUM_HEADS).transpose(2, 0, 1)  # (6, anc, tok)
    b2 = 16.0 / (1.0 + np.exp(-bt2[iw]))
    b2 = b2.reshape(N1, N2, NUM_HEADS).transpose(2, 0, 1)  # (6, tok, anc)

    expb = np.zeros((128, S_W), f32)
    for h in range(6):
        j = h % 3
        e = h // 3
        for t in (0, 1):
            expb[:, 512 * j + 128 * t + 64 * e:512 * j + 128 * t + 64 * e + 64] = \
                np.exp(b1[h, :, 128 * t:128 * (t + 1)]).T
        expb[64 * e:64 * e + 64, 512 * j + 256:512 * j + 512] = np.exp(b2[h]).T

    s1 = np.exp(np.minimum(np.asarray(logit_scale1, f32).reshape(NUM_HEADS), LOGIT_MAX))
    s2 = np.exp(np.minimum(np.asarray(logit_scale2, f32).reshape(NUM_HEADS), LOGIT_MAX))

    qkv4 = np.ascontiguousarray(np.asarray(qkv, f32).reshape(B, 16, STRIPE, 16, STRIPE, 3 * DIM)
                                .transpose(0, 1, 3, 2, 4, 5)).reshape(NWIN, N1, 3 * DIM)
    q = qkv4[:, :, :DIM].reshape(NWIN, N1, NUM_HEADS, HD)
    k = qkv4[:, :, DIM:2 * DIM].reshape(NWIN, N1, NUM_HEADS, HD)
    v = qkv4[:, :, 2 * DIM:].reshape(NWIN, N1, NUM_HEADS, HD)
    anc4 = np.ascontiguousarray(np.asarray(anchor, f32).reshape(B, 16, ANCH, 16, ANCH, DIM)
                                .transpose(0, 1, 3, 2, 4, 5)).reshape(NWIN, N2, NUM_HEADS, HD)

    kn = _l2n(k) * s1[None, None, :, None]
    qn = _l2n(q) * s2[None, None, :, None]
    an = _l2n(anc4)

    blob = np.zeros((NWIN, 128, BLOB_W), BF16)
    for h in range(6):
        r = 32 * (h % 3)
        cb = h // 3
        blob[:, r:r + 32, KT_OFF + 256 * cb:KT_OFF + 256 * cb + 256] = kn[:, :, h, :].transpose(0, 2, 1)
        blob[:, r:r + 32, QT_OFF + 256 * cb:QT_OFF + 256 * cb + 256] = qn[:, :, h, :].transpose(0, 2, 1)
        blob[:, r:r + 32, ANC_OFF + 64 * cb:ANC_OFF + 64 * cb + 64] = an[:, :, h, :].transpose(0, 2, 1)
    for j in range(3):
        for t in (0, 1):
            c0 = VO_OFF + 130 * j + 65 * t
            blob[:, :, c0] = 1.0
            blob[:, :, c0 + 1:c0 + 33] = v[:, 128 * t:128 * (t + 1), j, :]
            blob[:, :, c0 + 33:c0 + 65] = v[:, 128 * t:128 * (t + 1), j + 3, :]

    in_maps = []
    expb_bf = expb.astype(BF16)
    blob = blob.reshape(NWIN // CH, CH, 128, BLOB_W).transpose(0, 2, 1, 3)
    blob = np.ascontiguousarray(blob)  # (NWIN/CH, 128, CH, BLOB_W)
    cpc = WPC // CH
    for c in range(NCORES):
        in_maps.append({
            "blob": blob[c * cpc:(c + 1) * cpc],
            "expb": expb_bf,
        })
    return in_maps


def _assemble(results):
    # out: (WPC/CH, 128, CH, 396) per core; col = 132j + 66t + c
    outw = np.concatenate(
        [np.asarray(r["out"], np.float32).transpose(0, 2, 1, 3)
         .reshape(WPC, 128, 3, 2, 66) for r in results], axis=0)
    x = np.empty((NWIN, 2, 128, NUM_HEADS, HD), np.float32)  # (w, t, part, h, d)
    for j in range(3):
        x[:, :, :, j, :] = (outw[:, :, j, :, 0:32] /
                            outw[:, :, j, :, 32:33]).transpose(0, 2, 1, 3)
        x[:, :, :, j + 3, :] = (outw[:, :, j, :, 33:65] /
                                outw[:, :, j, :, 65:66]).transpose(0, 2, 1, 3)
    full = x.reshape(NWIN, N1, DIM)
    img = full.reshape(B, 16, 16, STRIPE, STRIPE, DIM).transpose(0, 1, 3, 2, 4, 5)
    return np.ascontiguousarray(img).reshape(B, HS * HS, DIM)


def _run(inputs, trace=False, trace_kwargs=None):
    in_maps = _prepare(
        inputs["qkv"], inputs["anchor"], inputs["table"],
        inputs["logit_scale1"], inputs["cpb1_w1"], inputs["cpb1_b1"], inputs["cpb1_w2"],
        inputs["logit_scale2"], inputs["cpb2_w1"], inputs["cpb2_b1"], inputs["cpb2_w2"],
        inputs["index_a2w"], inputs["index_w2a"],
    )
    nc = _get_nc()
    res = run_bass_kernel_spmd(
        nc, in_maps, core_ids=list(range(NCORES)),
        trace=trace, **(trace_kwargs or {}),
    )
    out = _assemble(res.results)
    return out, res


def kernel(**inputs):
    out, _ = _run(inputs, trace=False)
    return out
